# revision 51
# baseline (speedup 1.0000x reference)
"""Involution (B=4, C=256, H=W=56, K=7, G=16, reduction=4) on 8 trn2 NeuronCores.

v2 design — pixel-blocks on partitions, compact kernel values, PE-side
tap accumulation. ~115us HW vs the 299-446us shifted-slice baseline.

Sharding: 8 shards = (batch b in 0..3) x (w-half in 0..1); each core computes
a [256, 56, 28] output slab.

Per-core partition layout: p = 32*pb + 16*ch + g for pb in 0..3 (14-row
pixel blocks), ch in 0..1 (8-channel halves of a group), g in 0..15
(groups); free dims carry (c' in 0..7, rows, cols). Putting pixel blocks on
partitions is the key move: the per-pixel kernel values live COMPACTLY on
the same partition as the pixels they weight, so no 16x channel broadcast
is ever materialized.

Pipeline per core:
  1. stage1 (PE): t_ext = [relu(bn(W1 @ x)); ones] in bf16, [65, 1568];
     BN folded into W1/b1 on host; q-chunks alternate psum banks.
  2. per-tap compact kernel matmul (PE): for tap k, 4 matmuls (one per pb,
     m=32 at tile_position=(0,32pb)) produce psum [128, 392] = w[g, k, pix]
     replicated only 2x (for ch), NOT 16x. One small ACT copy -> bf16 SBUF
     (0.6us/tap instead of the 3.1us/tap the 16x-broadcast baseline paid).
  3. involution mul (DVE, the critical 49 x 1.78us stream): prod =
     x2win(i,j) * w2sb[:, k] where the w operand broadcasts over c' via a
     stride-0 AP dim — verified to stay in the DVE 2x perf mode, including
     odd (2-byte-aligned) window offsets.
  4. tap accumulation (PE): identity-matmuls accumulate prod into a 7-bank
     f32 PSUM accumulator (448-col chunks, start/stop accumulate flags).
     PE absorbs all adds under the DVE mul stream; f32 accumulation also
     improves accuracy vs bf16 trees (rel err ~5e-3).
  5. merge/out: the accumulator is 7 per-bank psum TILES, so each chunk's
     psum->bf16 copy (DVE chunks 0-2, ACT 3-6, in parallel) is gated only
     on that chunk's final add; 3 pipelined output DMAs follow.

Known sharp edges: gpsimd (Pool) tensor ops poison concurrent DVE
throughput (SBUF contention) — keep Pool idle; more/smaller DMAs lose to
queue issue overhead (~0.7us per dma_start on the issuing engine); HW runs
are bimodal — ~10-15% of executions land in a mode where every DVE
tensor_tensor runs ~20% slower (1.78 -> 2.15us, +~18us total) regardless
of code or SBUF addresses, so benchmark numbers need 2-3 samples.
"""

import numpy as np
import ml_dtypes
from contextlib import ExitStack

import concourse.bass as bass
import concourse.bacc as bacc
import concourse.tile as tile
from concourse import mybir
from concourse.bass_utils import run_bass_kernel_spmd

BF16 = ml_dtypes.bfloat16

B, C, H, W = 4, 256, 56, 56
KK, G, PAD = 7, 16, 3
Cr = 64
EPS = 1e-5
WH = W // 2               # 28 cols per w-half shard
NPIX = H * WH             # 1568 output pixels per shard
NPB = 4                   # pixel blocks (partition dim)
RB = H // NPB             # 14 rows per block
PS = RB * WH              # 392 pixels per block
XR, XC = RB + 2 * PAD, WH + 2 * PAD  # 20, 34 per-block padded window
NCORES = 8

# per-tap accumulation engine: 'PE' (psum f32), 'DVE' or 'POOL' (sbuf bf16)
ADD_ASSIGN = ['PE'] * 49

_CACHE = {}

# set by test.py to collect a hardware profile
TRACE = False
LAST_RESULT = None


def _build_nc():
    nc = bacc.Bacc("TRN2", target_bir_lowering=False, debug=False,
                   num_devices=NCORES)

    f32 = mybir.dt.float32
    bf16 = mybir.dt.bfloat16

    x2_d = nc.declare_dram_parameter("x2", [128, 8, XR, XC], bf16, isOutput=False)
    xd_d = nc.declare_dram_parameter("xd", [2, 128, NPIX], bf16, isOutput=False)
    w1t_d = nc.declare_dram_parameter("w1t", [2, 128, Cr], bf16, isOutput=False)
    b1p_d = nc.declare_dram_parameter("b1p", [Cr, 1], f32, isOutput=False)
    w2e_d = nc.declare_dram_parameter("w2e", [Cr + 1, 49, 32], bf16, isOutput=False)
    id_d = nc.declare_dram_parameter("ident", [128, 128], bf16, isOutput=False)
    out_d = nc.declare_dram_parameter("out", [128, 8, PS], bf16, isOutput=True)

    PE_TAPS = [k for k in range(49) if ADD_ASSIGN[k] == 'PE']
    DVE_TAPS = [k for k in range(49) if ADD_ASSIGN[k] == 'DVE']
    POOL_TAPS = [k for k in range(49) if ADD_ASSIGN[k] == 'POOL']
    # 7 uniform 448-col chunks; acc tile is [128, 7, 512] so each chunk
    # starts at a psum bank boundary (matmul dst must stay within a bank)
    ACC_CHUNKS = [(c, 448) for c in range(7)]

    with tile.TileContext(nc) as tc, ExitStack() as ctx:
        const = ctx.enter_context(tc.tile_pool(name="const", bufs=1))
        prodp = ctx.enter_context(tc.tile_pool(name="prod", bufs=5))
        accsb = ctx.enter_context(tc.tile_pool(name="accsb", bufs=1))
        outp = ctx.enter_context(tc.tile_pool(name="outp", bufs=1))
        psum_acc = ctx.enter_context(
            tc.tile_pool(name="psacc", bufs=1, space=bass.MemorySpace.PSUM))
        psum_w = ctx.enter_context(
            tc.tile_pool(name="psw", bufs=1, space=bass.MemorySpace.PSUM))

        # ---- constant / input loads (spread across DMA queues; x2/x2s are
        # chunked by row band so early taps can start before the full load) ----
        # stage1's inputs first: weights + xd split across both HW queues
        w1t_sb = const.tile([128, 2, Cr], bf16)
        nc.sync.dma_start(w1t_sb[:, 0, :], w1t_d[0])
        nc.scalar.dma_start(w1t_sb[:, 1, :], w1t_d[1])
        b1p_sb = const.tile([Cr, 1], f32)
        nc.sync.dma_start(b1p_sb[:], b1p_d[:])
        xd_sb = const.tile([128, 2, NPIX], bf16)
        nc.sync.dma_start(xd_sb[:, 0, :], xd_d[0])
        nc.scalar.dma_start(xd_sb[:, 1, :], xd_d[1])
        w2e_sb = const.tile([Cr + 1, 49, 32], bf16)
        nc.sync.dma_start(w2e_sb[:], w2e_d[:])
        id_sb = const.tile([128, 128], bf16)
        nc.sync.dma_start(id_sb[:], id_d[:])
        x2_sb = const.tile([128, 8, XR, XC], bf16)
        nc.scalar.dma_start(x2_sb[:, :, 0:7], x2_d[:, :, 0:7])
        nc.sync.dma_start(x2_sb[:, :, 7:14], x2_d[:, :, 7:14])
        nc.scalar.dma_start(x2_sb[:, :, 14:XR], x2_d[:, :, 14:XR])

        # ---- stage 1: t_ext = [relu(W1p @ x + b1p); ones] ----
        # odd q-chunks borrow an idle acc bank so consecutive chunks overlap.
        # acc is 7 per-bank tiles (not one tile) so each chunk's final reader
        # is gated only on that chunk's last add, not the whole accumulator.
        acc_t = [psum_acc.tile([128, 512], f32, tag=f"acc{c}", name=f"acc{c}")
                 for c in range(7)]
        t_ext = const.tile([Cr + 1, NPIX], bf16)
        nc.vector.memset(t_ext[Cr:Cr + 1, :], 1.0)

        def emit_s2(k, pw, pbs):
            for pb in pbs:
                nc.tensor.matmul(pw[32 * pb:32 * pb + 32, :],
                                 w2e_sb[:, k, :],
                                 t_ext[:, pb * PS:(pb + 1) * PS],
                                 start=True, stop=True,
                                 tile_position=(0, 32 * pb))

        # tap-0's per-pb kernel matmuls interleave into stage1: each needs
        # only its own relu chunk, so the first tap's weights are ready
        # almost immediately after stage1 finishes
        pw0 = psum_w.tile([128, PS], f32, tag="pw", name="pw0")
        for q in range(NPB):
            pt = acc_t[q][0:Cr, 0:PS]
            for chh in range(2):
                nc.tensor.matmul(pt, w1t_sb[:, chh, :],
                                 xd_sb[:, chh, q * PS:(q + 1) * PS],
                                 start=(chh == 0), stop=(chh == 1),
                                 skip_group_check=True)
            nc.scalar.activation(t_ext[0:Cr, q * PS:(q + 1) * PS], pt,
                                 mybir.ActivationFunctionType.Relu,
                                 bias=b1p_sb[:], scale=1.0)
            if q >= 1:
                emit_s2(0, pw0[:], [q - 1])
        emit_s2(0, pw0[:], [3])

        # ---- per-tap pipeline ----
        w2sb = const.tile([128, 49, PS], bf16)
        accD = accsb.tile([128, 3136], bf16)
        accP = accsb.tile([128, 3136], bf16)
        if POOL_TAPS:
            nc.gpsimd.memset(accP[:], 0.0)

        prods = {}
        nD = 0

        def issue_add(k):
            nonlocal nD
            eng = ADD_ASSIGN[k]
            pr = prods.pop(k)
            prf = pr[:].rearrange("p a b c -> p (a b c)")
            if eng == 'PE':
                first = (k == PE_TAPS[0])
                last = (k == PE_TAPS[-1])
                for (c, n) in ACC_CHUNKS:
                    nc.tensor.matmul(acc_t[c][:, 0:n], id_sb[:],
                                     prf[:, c * n:(c + 1) * n],
                                     start=first, stop=last,
                                     skip_group_check=True)
            elif eng == 'DVE':
                if nD == 0:
                    nc.vector.tensor_copy(accD[:], prf)
                else:
                    nc.vector.tensor_add(accD[:], accD[:], prf)
                nD += 1
            else:
                nc.gpsimd.tensor_add(accP[:], accP[:], prf)

        LAG = 3
        for k in range(49):
            i, j = k // KK, k % KK
            # stage 2: compact kernel values for tap k -> psum_w
            if k == 0:
                pw = pw0
            else:
                pw = psum_w.tile([128, PS], f32, tag="pw")
                emit_s2(k, pw[:], range(NPB))
            nc.scalar.copy(w2sb[:, k, :], pw[:])
            # involution multiply for tap k
            wb = (w2sb[:, k, :].rearrange("p (r c) -> p r c", r=RB)
                  .unsqueeze(1).broadcast_to([128, 8, RB, WH]))
            xwin = x2_sb[:, :, i:i + RB, j:j + WH]
            pr = prodp.tile([128, 8, RB, WH], bf16, tag="pr")
            nc.vector.tensor_mul(pr[:], xwin, wb)
            prods[k] = pr
            # lagged accumulation keeps PE's stage-2 ahead of the adds
            if k >= LAG:
                issue_add(k - LAG)
        for k in range(49 - LAG, 49):
            issue_add(k)

        # ---- merge: per-chunk psum->sbuf copies (DVE+ACT parallel), then
        # three pipelined output DMAs gated only on their own chunks ----
        if DVE_TAPS and POOL_TAPS:
            nc.vector.tensor_add(accD[:], accD[:], accP[:])
        sb_parts = accD if DVE_TAPS else (accP if POOL_TAPS else None)
        of = outp.tile([128, 3136], bf16)
        odv = out_d[:].rearrange("p a b -> p (a b)")
        for c in range(7):
            dst = of[:, c * 448:(c + 1) * 448]
            if c < 4:
                nc.vector.tensor_copy(dst, acc_t[c][:, 0:448])
            else:
                nc.scalar.copy(dst, acc_t[c][:, 0:448])
            if sb_parts is not None:
                nc.vector.tensor_add(dst, dst, sb_parts[:, c * 448:(c + 1) * 448])
        for pi, (c0, c1) in enumerate(((0, 4), (4, 7))):
            cs = slice(c0 * 448, c1 * 448)
            eng = nc.sync if pi == 0 else nc.scalar
            eng.dma_start(odv[:, cs], of[:, cs])

    nc.compile()
    return nc


def _prep_host_inputs(inputs, W1, b1, gamma, beta, mean, var, W2, b2):
    scale = gamma / np.sqrt(var + EPS)
    shift = beta - mean * scale
    W1p = W1 * scale[:, None]
    b1p = (b1 * scale + shift).astype(np.float32).reshape(Cr, 1)
    w1t = np.ascontiguousarray(W1p.T.reshape(2, 128, Cr)).astype(BF16)

    # stage-2 lhsT: [65, 49, 32], columns (16*ch + g) duplicated over ch
    W2r = W2.reshape(G, KK * KK, Cr)       # [g, k, m]
    b2r = b2.reshape(G, KK * KK)
    w2e = np.zeros((Cr + 1, 49, 32), np.float32)
    for ch in range(2):
        w2e[0:Cr, :, 16 * ch:16 * ch + 16] = np.transpose(W2r, (2, 1, 0))
        w2e[Cr, :, 16 * ch:16 * ch + 16] = b2r.T
    w2e = w2e.astype(BF16)

    ident = np.eye(128, dtype=np.float32).astype(BF16)

    xb = np.asarray(inputs, np.float32)
    xp = np.pad(xb, ((0, 0), (0, 0), (PAD, PAD), (PAD, PAD)))

    per_core = []
    for core in range(NCORES):
        b, wh = core // 2, core % 2
        x2 = np.zeros((128, 8, XR, XC), BF16)
        base = xp[b]                        # [256, 62, 62]
        c0 = wh * WH                        # global col offset of this half
        for pb in range(NPB):
            rows = slice(RB * pb, RB * pb + XR)
            blk = base[:, rows, c0:c0 + XC].astype(BF16)     # [256, 20, 34]
            # channel -> (g, ch, c'); partition = 32*pb + 16*ch + g
            for ch in range(2):
                for g in range(G):
                    p = 32 * pb + 16 * ch + g
                    cidx = 16 * g + 8 * ch
                    x2[p] = blk[cidx:cidx + 8]
        xd = np.ascontiguousarray(
            xb[b, :, :, c0:c0 + WH].reshape(2, 128, NPIX)).astype(BF16)
        per_core.append({"x2": x2, "xd": xd, "w1t": w1t,
                         "b1p": b1p, "w2e": w2e, "ident": ident})
    return per_core


def kernel(inputs, W1, b1, gamma, beta, mean, var, W2, b2):
    global LAST_RESULT
    inputs = np.asarray(inputs, np.float32)
    if "nc" not in _CACHE:
        _CACHE["nc"] = _build_nc()
    nc = _CACHE["nc"]

    in_maps = _prep_host_inputs(
        inputs, np.asarray(W1, np.float32), np.asarray(b1, np.float32),
        np.asarray(gamma, np.float32), np.asarray(beta, np.float32),
        np.asarray(mean, np.float32), np.asarray(var, np.float32),
        np.asarray(W2, np.float32), np.asarray(b2, np.float32))

    res = run_bass_kernel_spmd(nc, in_maps, list(range(NCORES)), trace=TRACE)
    LAST_RESULT = res

    out = np.empty((B, C, H, W), np.float32)
    for core in range(NCORES):
        b, wh = core // 2, core % 2
        o = res.results[core]["out"].reshape(4, 2, G, 8, RB, WH)
        # (pb, ch, g, c', r, c) -> channel (g, ch, c'), row (pb, r)
        o = o.transpose(2, 1, 3, 0, 4, 5).reshape(C, H, WH)
        out[b, :, :, wh * WH:(wh + 1) * WH] = o
    return out
